# revision 48
# baseline (speedup 1.0000x reference)
"""Multi-head attention Trainium2 kernel, SPMD over 8 NeuronCores.

Problem: x:(2,1024,1024), Wq:(1024,1024), Wkv:(2048,1024), Wp:(1024,1024).
reference returns (out, attn) with attn:(b, n, m, h) softmax probabilities.

Sharding: tensor-parallel over heads — 2 heads (= a 128-wide feature slice)
per core. Each core computes Q^T/K^T/V for its heads over all tokens, the
full attention for its heads (writing its slice of `attn`), and a partial
output projection (row-sharded Wp); partials are summed on the host during
the gather.

All matmuls run in bf16 (fp32 PSUM accumulation). The softmax runs without
max-subtraction (|scores*scale| <~ 6 for this problem family; exp is safe in
fp32): E = exp(S/8) on ScalarE straight out of PSUM. The attn@V matmul uses
a 65-column V_aug = [V_h | ones] so its PSUM accumulator also carries the
softmax row-sums D (row 64). Normalization uses reciprocal_approx_fast on a
partition-broadcast of D and DVE multiplies.

The emission order interleaves projections with attention groups so the
ScalarE exp stream starts ~10us in and every engine stays busy:
K(b0) -> Q(b0) -> scores/exp(b0) -> V -> attn@V(b0) -> K/Q(b1) -> ...
"""

import numpy as np
import ml_dtypes

import concourse.bass as bass
import concourse.tile as tile
import concourse.mybir as mybir
from concourse import bacc
from concourse.bass_utils import run_bass_kernel_spmd
from concourse.masks import make_identity

BF16 = mybir.dt.bfloat16
F32 = mybir.dt.float32
AF = mybir.ActivationFunctionType

B = 2
N = 1024  # tokens per batch
C = 1024  # model dim
H = 16
DH = 64
CORES = 8
T = B * N  # 2048 flattened tokens
KT = C // 128  # 8 contraction tiles
SCALE = DH**-0.5  # 0.125

_NC_CACHE = {}


def build_nc():
    if "nc" in _NC_CACHE:
        return _NC_CACHE["nc"]
    nc = bacc.Bacc("TRN2", target_bir_lowering=False, debug=False, num_devices=CORES)

    # inputs come pre-laid-out from the host so every load is a contiguous
    # [128, X] DMA: xt[p, k, t] = x[t, k*128+p], w*[p, k, f] = W[f, k*128+p]
    xt = nc.declare_dram_parameter("xt", [128, KT, T], BF16, isOutput=False)
    wq = nc.declare_dram_parameter("wq", [128, KT, 128], BF16, isOutput=False)
    wk = nc.declare_dram_parameter("wk", [128, KT, 128], BF16, isOutput=False)
    wv = nc.declare_dram_parameter("wv", [128, KT, 128], BF16, isOutput=False)
    wp = nc.declare_dram_parameter("wp", [128, C], BF16, isOutput=False)
    bq = nc.declare_dram_parameter("bq", [128, 1], F32, isOutput=False)
    bk = nc.declare_dram_parameter("bk", [128, 1], F32, isOutput=False)
    bv = nc.declare_dram_parameter("bv", [128, 1], F32, isOutput=False)
    # attn slice for this core's 2 heads, stored (b, h_local, m, n)
    attn_o = nc.declare_dram_parameter("attn_o", [B, 2, N, N], BF16, isOutput=True)
    # partial output projection (t, e); host sums over cores
    out_p = nc.declare_dram_parameter("out_p", [T, C], BF16, isOutput=True)

    with tile.TileContext(nc) as tc:
        with (
            tc.tile_pool(name="const", bufs=1) as constp,
            tc.tile_pool(name="xw", bufs=1) as xw,
            tc.tile_pool(name="qkv", bufs=1) as qkv,
            tc.tile_pool(name="epool", bufs=4) as epool,
            tc.tile_pool(name="norm", bufs=3) as norm,
            tc.tile_pool(name="obuf", bufs=1) as obuf,
            tc.tile_pool(name="stage", bufs=4) as stage,
            tc.tile_pool(name="ps_sc", bufs=2, space="PSUM") as ps_sc,
            tc.tile_pool(name="ps_acc", bufs=1, space="PSUM") as ps_acc,
        ):
            # ---- constants ----
            ident = constp.tile([128, 128], BF16)
            make_identity(nc, ident[:])
            ones_bc = constp.tile([128, 128], BF16)
            nc.gpsimd.memset(ones_bc[:], 1.0)

            # ---- input loads (all contiguous [128, X] descriptors) ----
            wk_sb = xw.tile([128, KT, 128], BF16)
            nc.sync.dma_start(wk_sb[:], wk[:])
            wq_sb = xw.tile([128, KT, 128], BF16)
            nc.scalar.dma_start(wq_sb[:], wq[:])
            bq_sb = xw.tile([128, 1], F32)
            nc.scalar.dma_start(bq_sb[:], bq[:])
            bk_sb = xw.tile([128, 1], F32)
            nc.scalar.dma_start(bk_sb[:], bk[:])
            xt_sb = xw.tile([128, KT, T], BF16)
            for k in range(KT):
                eng = (nc.sync, nc.scalar, nc.gpsimd)[k % 3]
                eng.dma_start(xt_sb[:, k, :], xt[:, k, :])
            wv_sb = xw.tile([128, KT, 128], BF16)
            nc.gpsimd.dma_start(wv_sb[:], wv[:])
            bv_sb = xw.tile([128, 1], F32)
            nc.gpsimd.dma_start(bv_sb[:], bv[:])
            wp_sb = xw.tile([128, C], BF16)
            nc.gpsimd.dma_start(wp_sb[:], wp[:])

            q_sb = qkv.tile([128, T], BF16)
            k_sb = qkv.tile([128, T], BF16)
            vt_sb = qkv.tile([128, T], BF16)  # V^T (dv, t), pre-transpose
            # V_aug per m-tile: [V_h0(64) | ones | V_h1(64) | ones] so the
            # attn@V matmul (M=65) also accumulates the softmax row-sums D
            v_sb = qkv.tile([128, 16, 130], BF16)
            nc.gpsimd.memset(v_sb[:], 1.0)
            o_sbs = [obuf.tile([128, N], BF16, name=f"o_sb{b}") for b in range(B)]

            def proj(w_t, b_t, dst, tgs):
                """project tile-groups tgs of tokens into dst (feat x T).

                k is the outer loop so each xt chunk is consumed as soon as
                its DMA lands (the QKV phase streams against the x load).
                """
                psqs = [
                    ps_sc.tile([128, 512], F32, name=f"qkv_ps{i}", tag="mix")
                    for i in range(len(tgs))
                ]
                for k in range(KT):
                    for i, tg in enumerate(tgs):
                        nc.tensor.matmul(
                            psqs[i][:],
                            w_t[:, k, :],
                            xt_sb[:, k, tg * 512 : (tg + 1) * 512],
                            start=(k == 0),
                            stop=(k == KT - 1),
                        )
                for i, tg in enumerate(tgs):
                    nc.vector.tensor_scalar_add(
                        dst[:, tg * 512 : (tg + 1) * 512], psqs[i][:], b_t[:]
                    )

            def v_transpose(tiles):
                for i in tiles:
                    vt_ps = ps_sc.tile([128, 128], BF16, name="vt_ps", tag="mix")
                    nc.tensor.transpose(
                        vt_ps[:], vt_sb[:, i * 128 : (i + 1) * 128], ident[:]
                    )
                    nc.vector.tensor_copy(v_sb[:, i, 0:64], vt_ps[:, 0:64])
                    nc.vector.tensor_copy(v_sb[:, i, 65:129], vt_ps[:, 64:128])

            e_ts = {}

            def scores_exp(b, ng):
                """scores + exp for all 8 m-tiles of group (b, ng)."""
                n0 = b * N + ng * 512
                e_t = epool.tile([128, 8, 1024], BF16, name="e_sb")
                e_ts[(b, ng)] = e_t
                for mt in range(8):
                    m0 = b * 8 + mt
                    scp = ps_sc.tile([128, 1024], F32, name="sc_ps")
                    for hl in range(2):
                        nc.tensor.matmul(
                            scp[:, hl * 512 : (hl + 1) * 512],
                            k_sb[hl * 64 : (hl + 1) * 64, m0 * 128 : (m0 + 1) * 128],
                            q_sb[hl * 64 : (hl + 1) * 64, n0 : n0 + 512],
                            start=True,
                            stop=True,
                        )
                    nc.scalar.activation(e_t[:, mt, :], scp[:], AF.Exp, scale=SCALE)

            def attnv_epilogue(b, ng):
                """attn@V + row-sum extraction + normalize + writes."""
                e_t = e_ts[(b, ng)]
                o0 = ps_acc.tile([65, 512], F32, name="o0")
                o1 = ps_acc.tile([65, 512], F32, name="o1")
                for mt in range(8):
                    m0 = b * 8 + mt
                    first, last = mt == 0, mt == 7
                    nc.tensor.matmul(
                        o0[:],
                        v_sb[:, m0, 0:65],
                        e_t[:, mt, 0:512],
                        start=first,
                        stop=last,
                    )
                    nc.tensor.matmul(
                        o1[:],
                        v_sb[:, m0, 65:130],
                        e_t[:, mt, 512:1024],
                        start=first,
                        stop=last,
                    )
                # extract D rows (psum part 64) to a base-0 partition,
                # broadcast to all 128, reciprocal
                dsb = norm.tile([128, 1024], BF16, name="dsb")
                nc.scalar.copy(dsb[64:65, 0:512], o0[64:65, :])
                nc.scalar.copy(dsb[64:65, 512:1024], o1[64:65, :])
                # broadcast D to all 128 partitions with a K=1 ones matmul
                # (lhsT sliced at base 64 to lane-align with the D rows)
                db_ps0 = ps_sc.tile([128, 512], F32, name="db_ps0", tag="mix")
                nc.tensor.matmul(
                    db_ps0[:], ones_bc[64:65, :], dsb[64:65, 0:512],
                    start=True, stop=True,
                )
                db_ps1 = ps_sc.tile([128, 512], F32, name="db_ps1", tag="mix")
                nc.tensor.matmul(
                    db_ps1[:], ones_bc[64:65, :], dsb[64:65, 512:1024],
                    start=True, stop=True,
                )
                rb = norm.tile([128, 1024], F32, name="rb")
                nc.vector.reciprocal_approx_fast(out=rb[:, 0:512], in_=db_ps0[:])
                nc.vector.reciprocal_approx_fast(out=rb[:, 512:1024], in_=db_ps1[:])
                rbb = norm.tile([128, 1024], BF16, name="rbb")
                nc.scalar.copy(rbb[:], rb[:])
                # O^T scaled: o_sb rows 0:64 <- h0; h1 lands at psum parts
                # 0:64, scaled into a staging tile then DMA-shifted to rows
                # 64:128
                nc.vector.tensor_mul(
                    o_sbs[b][0:64, ng * 512 : (ng + 1) * 512],
                    o0[0:64, :],
                    rb[0:64, 0:512],
                )
                otmp = norm.tile([64, 512], BF16, name="otmp")
                nc.vector.tensor_mul(otmp[:], o1[0:64, :], rb[0:64, 512:1024])
                nc.scalar.dma_start(
                    o_sbs[b][64:128, ng * 512 : (ng + 1) * 512], otmp[:]
                )
                # normalize E in place (both heads in one op per mt) and
                # write the attn slice (both heads in one DMA per mt)
                for mt in range(8):
                    nc.vector.tensor_mul(e_t[:, mt, :], e_t[:, mt, :], rbb[:])
                    dst = attn_o[
                        b, :, mt * 128 : (mt + 1) * 128, ng * 512 : (ng + 1) * 512
                    ].rearrange("h m n -> m h n")
                    src = e_t[:, mt, :].rearrange("m (h n) -> m h n", h=2)
                    nc.sync.dma_start(dst, src)

            def out_proj(b, tts):
                """partial output projection for token tiles tts of batch b.
                Tile tt only needs o_sb columns tt*128:(tt+1)*128, so the
                first half can run right after the (b, ng=0) epilogue."""
                for tt in tts:
                    st = stage.tile([128, 1024], BF16, name="op_st")
                    for eg in range(2):
                        op_ps = ps_sc.tile([128, 512], F32, name="op_ps", tag="mix")
                        nc.tensor.matmul(
                            op_ps[:],
                            o_sbs[b][:, tt * 128 : (tt + 1) * 128],
                            wp_sb[:, eg * 512 : (eg + 1) * 512],
                            start=True,
                            stop=True,
                        )
                        if eg == 0:
                            nc.scalar.copy(st[:, 0:512], op_ps[:])
                        else:
                            nc.vector.tensor_copy(st[:, 512:1024], op_ps[:])
                    nc.gpsimd.dma_start(
                        out_p[(b * 8 + tt) * 128 : (b * 8 + tt + 1) * 128, :],
                        st[:],
                    )

            # ---- interleaved schedule ----
            proj(wk_sb, bk_sb, k_sb, [0, 1])  # K for b0
            proj(wq_sb, bq_sb, q_sb, [0])  # Q for b0/ng0
            scores_exp(0, 0)
            proj(wq_sb, bq_sb, q_sb, [1])  # Q for b0/ng1
            scores_exp(0, 1)
            proj(wv_sb, bv_sb, vt_sb, [0, 1])  # V for b0
            v_transpose(range(0, 8))
            attnv_epilogue(0, 0)
            proj(wk_sb, bk_sb, k_sb, [2, 3])  # K for b1
            proj(wq_sb, bq_sb, q_sb, [2, 3])  # Q for b1
            attnv_epilogue(0, 1)
            scores_exp(1, 0)
            proj(wv_sb, bv_sb, vt_sb, [2, 3])  # V for b1
            v_transpose(range(8, 16))
            scores_exp(1, 1)
            attnv_epilogue(1, 0)
            out_proj(0, range(0, 8))
            out_proj(1, range(0, 4))
            attnv_epilogue(1, 1)
            out_proj(1, range(4, 8))

    nc.compile()
    _NC_CACHE["nc"] = nc
    return nc


def prepare_in_maps(x, Wq, bq, Wkv, bkv, Wp, bp):
    bf = ml_dtypes.bfloat16
    x = np.asarray(x, np.float32)
    Wq = np.asarray(Wq, np.float32)
    Wkv = np.asarray(Wkv, np.float32)
    Wp = np.asarray(Wp, np.float32)
    bq = np.asarray(bq, np.float32)
    bkv = np.asarray(bkv, np.float32)

    # xt[p, k, t] = x[t, k*128+p]
    xt = np.ascontiguousarray(x.reshape(T, KT, 128).transpose(2, 1, 0)).astype(bf)

    def wprep(Wslice):  # [128 f, C] -> [p, k, f] with c = k*128+p
        return np.ascontiguousarray(
            Wslice.T.reshape(KT, 128, 128).transpose(1, 0, 2)
        ).astype(bf)

    in_maps = []
    for core in range(CORES):
        f0 = core * 128
        in_maps.append(
            {
                "xt": xt,
                "wq": wprep(Wq[f0 : f0 + 128, :]),
                "wk": wprep(Wkv[f0 : f0 + 128, :]),
                "wv": wprep(Wkv[C + f0 : C + f0 + 128, :]),
                "wp": np.ascontiguousarray(Wp[:, f0 : f0 + 128].T).astype(bf),
                "bq": bq[f0 : f0 + 128].reshape(128, 1).astype(np.float32),
                "bk": bkv[f0 : f0 + 128].reshape(128, 1).astype(np.float32),
                "bv": bkv[C + f0 : C + f0 + 128].reshape(128, 1).astype(np.float32),
            }
        )
    return in_maps


def gather_outputs(results, bp):
    bp = np.asarray(bp, np.float32)
    attn = np.empty((B, N, N, H), np.float32)
    out = np.zeros((T, C), np.float32)
    for core in range(CORES):
        r = results[core]
        a = r["attn_o"]  # [B, 2, m, n] bf16
        for hl in range(2):
            attn[:, :, :, core * 2 + hl] = a[:, hl].astype(np.float32).transpose(
                0, 2, 1
            )
        out += r["out_p"].astype(np.float32)
    out += bp[None, :]
    return out.reshape(B, N, C), attn


def kernel(x, Wq, bq, Wkv, bkv, Wp, bp):
    nc = build_nc()
    in_maps = prepare_in_maps(x, Wq, bq, Wkv, bkv, Wp, bp)
    res = run_bass_kernel_spmd(nc, in_maps, core_ids=list(range(CORES)))
    return gather_outputs(res.results, bp)


# revision 49
# speedup vs baseline: 1.2187x; 1.2187x over previous
"""Multi-head attention Trainium2 kernel, SPMD over 8 NeuronCores.

Problem: x:(2,1024,1024), Wq:(1024,1024), Wkv:(2048,1024), Wp:(1024,1024).
reference returns (out, attn) with attn:(b, n, m, h) softmax probabilities.

Sharding: tensor-parallel over heads — 2 heads (= a 128-wide feature slice)
per core. Each core computes Q^T/K^T/V for its heads over all tokens, the
full attention for its heads (writing its slice of `attn`), and a partial
output projection (row-sharded Wp); partials are summed on the host during
the gather.

All matmuls run in bf16 (fp32 PSUM accumulation). The softmax runs without
max-subtraction (|scores*scale| <~ 6 for this problem family; exp is safe in
fp32): E = exp(S/8) on ScalarE straight out of PSUM. The attn@V matmul uses
a 65-column V_aug = [V_h | ones] so its PSUM accumulator also carries the
softmax row-sums D (row 64). Normalization uses reciprocal_approx_fast on a
partition-broadcast of D and DVE multiplies.

The emission order interleaves projections with attention groups so the
ScalarE exp stream starts ~10us in and every engine stays busy:
K(b0) -> Q(b0) -> scores/exp(b0) -> V -> attn@V(b0) -> K/Q(b1) -> ...
"""

import numpy as np
import ml_dtypes

import concourse.bass as bass
import concourse.tile as tile
import concourse.mybir as mybir
from concourse import bacc
from concourse.bass_utils import run_bass_kernel_spmd
from concourse.masks import make_identity

BF16 = mybir.dt.bfloat16
F32 = mybir.dt.float32
AF = mybir.ActivationFunctionType

B = 2
N = 1024  # tokens per batch
C = 1024  # model dim
H = 16
DH = 64
CORES = 8
T = B * N  # 2048 flattened tokens
KT = C // 128  # 8 contraction tiles
SCALE = DH**-0.5  # 0.125

_NC_CACHE = {}


def build_nc():
    if "nc" in _NC_CACHE:
        return _NC_CACHE["nc"]
    nc = bacc.Bacc("TRN2", target_bir_lowering=False, debug=False, num_devices=CORES)

    # inputs come pre-laid-out from the host so every load is a contiguous
    # [128, X] DMA: xt[p, k, t] = x[t, k*128+p], w*[p, k, f] = W[f, k*128+p]
    xt = nc.declare_dram_parameter("xt", [128, KT, T], BF16, isOutput=False)
    wq = nc.declare_dram_parameter("wq", [128, KT, 128], BF16, isOutput=False)
    wk = nc.declare_dram_parameter("wk", [128, KT, 128], BF16, isOutput=False)
    wv = nc.declare_dram_parameter("wv", [128, KT, 128], BF16, isOutput=False)
    wp = nc.declare_dram_parameter("wp", [128, C], BF16, isOutput=False)
    bq = nc.declare_dram_parameter("bq", [128, 1], F32, isOutput=False)
    bk = nc.declare_dram_parameter("bk", [128, 1], F32, isOutput=False)
    bv = nc.declare_dram_parameter("bv", [128, 1], F32, isOutput=False)
    # attn slice for this core's 2 heads, stored (b, h_local, m, n)
    attn_o = nc.declare_dram_parameter("attn_o", [B, 2, N, N], BF16, isOutput=True)
    # partial output projection (t, e); host sums over cores
    out_p = nc.declare_dram_parameter("out_p", [T, C], BF16, isOutput=True)

    with tile.TileContext(nc) as tc:
        with (
            tc.tile_pool(name="const", bufs=1) as constp,
            tc.tile_pool(name="xw", bufs=1) as xw,
            tc.tile_pool(name="qkv", bufs=1) as qkv,
            tc.tile_pool(name="epool", bufs=4) as epool,
            tc.tile_pool(name="norm", bufs=3) as norm,
            tc.tile_pool(name="obuf", bufs=1) as obuf,
            tc.tile_pool(name="stage", bufs=4) as stage,
            tc.tile_pool(name="ps_sc", bufs=2, space="PSUM") as ps_sc,
            tc.tile_pool(name="ps_acc", bufs=1, space="PSUM") as ps_acc,
        ):
            # ---- constants ----
            ident = constp.tile([128, 128], BF16)
            make_identity(nc, ident[:])
            ones_bc = constp.tile([128, 128], BF16)
            nc.gpsimd.memset(ones_bc[:], 1.0)

            # ---- input loads (all contiguous [128, X] descriptors) ----
            wk_sb = xw.tile([128, KT, 128], BF16)
            nc.sync.dma_start(wk_sb[:], wk[:])
            wq_sb = xw.tile([128, KT, 128], BF16)
            nc.scalar.dma_start(wq_sb[:], wq[:])
            bq_sb = xw.tile([128, 1], F32)
            nc.scalar.dma_start(bq_sb[:], bq[:])
            bk_sb = xw.tile([128, 1], F32)
            nc.scalar.dma_start(bk_sb[:], bk[:])
            xt_sb = xw.tile([128, KT, T], BF16)
            for k in range(KT):
                eng = (nc.sync, nc.scalar, nc.gpsimd)[k % 3]
                eng.dma_start(xt_sb[:, k, :], xt[:, k, :])
            wv_sb = xw.tile([128, KT, 128], BF16)
            nc.gpsimd.dma_start(wv_sb[:], wv[:])
            bv_sb = xw.tile([128, 1], F32)
            nc.gpsimd.dma_start(bv_sb[:], bv[:])
            wp_sb = xw.tile([128, C], BF16)
            nc.gpsimd.dma_start(wp_sb[:], wp[:])

            q_sb = qkv.tile([128, T], BF16)
            k_sb = qkv.tile([128, T], BF16)
            vt_sb = qkv.tile([128, T], BF16)  # V^T (dv, t), pre-transpose
            # V_aug per m-tile: [V_h0(64) | ones | V_h1(64) | ones] so the
            # attn@V matmul (M=65) also accumulates the softmax row-sums D
            v_sb = qkv.tile([128, 16, 130], BF16)
            nc.gpsimd.memset(v_sb[:], 1.0)
            o_sbs = [obuf.tile([128, N], BF16, name=f"o_sb{b}") for b in range(B)]

            def proj(w_t, b_t, dst, tgs):
                """project tile-groups tgs of tokens into dst (feat x T).

                k is the outer loop so each xt chunk is consumed as soon as
                its DMA lands (the QKV phase streams against the x load).
                """
                psqs = [
                    ps_sc.tile([128, 512], F32, name=f"qkv_ps{i}", tag="mix")
                    for i in range(len(tgs))
                ]
                for k in range(KT):
                    for i, tg in enumerate(tgs):
                        nc.tensor.matmul(
                            psqs[i][:],
                            w_t[:, k, :],
                            xt_sb[:, k, tg * 512 : (tg + 1) * 512],
                            start=(k == 0),
                            stop=(k == KT - 1),
                        )
                for i, tg in enumerate(tgs):
                    nc.vector.tensor_scalar_add(
                        dst[:, tg * 512 : (tg + 1) * 512], psqs[i][:], b_t[:]
                    )

            def v_transpose(tiles):
                for i in tiles:
                    vt_ps = ps_sc.tile([128, 128], BF16, name="vt_ps", tag="mix")
                    nc.tensor.transpose(
                        vt_ps[:], vt_sb[:, i * 128 : (i + 1) * 128], ident[:]
                    )
                    nc.vector.tensor_copy(v_sb[:, i, 0:64], vt_ps[:, 0:64])
                    nc.vector.tensor_copy(v_sb[:, i, 65:129], vt_ps[:, 64:128])

            e_ts = {}

            def scores_exp(b, ng):
                """scores + exp for all 8 m-tiles of group (b, ng)."""
                n0 = b * N + ng * 512
                e_t = epool.tile([128, 8, 1024], BF16, name="e_sb")
                e_ts[(b, ng)] = e_t
                for mt in range(8):
                    m0 = b * 8 + mt
                    scp = ps_sc.tile([128, 1024], F32, name="sc_ps")
                    for hl in range(2):
                        nc.tensor.matmul(
                            scp[:, hl * 512 : (hl + 1) * 512],
                            k_sb[hl * 64 : (hl + 1) * 64, m0 * 128 : (m0 + 1) * 128],
                            q_sb[hl * 64 : (hl + 1) * 64, n0 : n0 + 512],
                            start=True,
                            stop=True,
                        )
                    nc.scalar.activation(e_t[:, mt, :], scp[:], AF.Exp, scale=SCALE)

            def attnv_epilogue(b, ng):
                """attn@V + row-sum extraction + normalize + writes."""
                e_t = e_ts[(b, ng)]
                o0 = ps_acc.tile([65, 512], F32, name="o0")
                o1 = ps_acc.tile([65, 512], F32, name="o1")
                for mt in range(8):
                    m0 = b * 8 + mt
                    first, last = mt == 0, mt == 7
                    nc.tensor.matmul(
                        o0[:],
                        v_sb[:, m0, 0:65],
                        e_t[:, mt, 0:512],
                        start=first,
                        stop=last,
                    )
                    nc.tensor.matmul(
                        o1[:],
                        v_sb[:, m0, 65:130],
                        e_t[:, mt, 512:1024],
                        start=first,
                        stop=last,
                    )
                # extract D rows (psum part 64) to a base-0 partition,
                # broadcast to all 128, reciprocal
                dsb = norm.tile([128, 1024], BF16, name="dsb")
                nc.vector.tensor_copy(dsb[64:65, 0:512], o0[64:65, :])
                nc.vector.tensor_copy(dsb[64:65, 512:1024], o1[64:65, :])
                # broadcast D to all 128 partitions with a K=1 ones matmul
                # (lhsT sliced at base 64 to lane-align with the D rows)
                db_ps0 = ps_sc.tile([128, 512], F32, name="db_ps0", tag="mix")
                nc.tensor.matmul(
                    db_ps0[:], ones_bc[64:65, :], dsb[64:65, 0:512],
                    start=True, stop=True,
                )
                db_ps1 = ps_sc.tile([128, 512], F32, name="db_ps1", tag="mix")
                nc.tensor.matmul(
                    db_ps1[:], ones_bc[64:65, :], dsb[64:65, 512:1024],
                    start=True, stop=True,
                )
                rb = norm.tile([128, 1024], F32, name="rb")
                nc.vector.reciprocal_approx_fast(out=rb[:, 0:512], in_=db_ps0[:])
                nc.vector.reciprocal_approx_fast(out=rb[:, 512:1024], in_=db_ps1[:])
                rbb = norm.tile([128, 1024], BF16, name="rbb")
                nc.vector.tensor_copy(rbb[:], rb[:])
                # O^T scaled: o_sb rows 0:64 <- h0; h1 lands at psum parts
                # 0:64, scaled into a staging tile then DMA-shifted to rows
                # 64:128
                nc.vector.tensor_mul(
                    o_sbs[b][0:64, ng * 512 : (ng + 1) * 512],
                    o0[0:64, :],
                    rb[0:64, 0:512],
                )
                otmp = norm.tile([64, 512], BF16, name="otmp")
                nc.vector.tensor_mul(otmp[:], o1[0:64, :], rb[0:64, 512:1024])
                nc.scalar.dma_start(
                    o_sbs[b][64:128, ng * 512 : (ng + 1) * 512], otmp[:]
                )
                # normalize E in place (both heads in one op per mt) and
                # write the attn slice (both heads in one DMA per mt)
                for mt in range(8):
                    nc.vector.tensor_mul(e_t[:, mt, :], e_t[:, mt, :], rbb[:])
                    dst = attn_o[
                        b, :, mt * 128 : (mt + 1) * 128, ng * 512 : (ng + 1) * 512
                    ].rearrange("h m n -> m h n")
                    src = e_t[:, mt, :].rearrange("m (h n) -> m h n", h=2)
                    nc.sync.dma_start(dst, src)

            def out_proj(b, tts):
                """partial output projection for token tiles tts of batch b.
                Tile tt only needs o_sb columns tt*128:(tt+1)*128, so the
                first half can run right after the (b, ng=0) epilogue."""
                for tt in tts:
                    st = stage.tile([128, 1024], BF16, name="op_st")
                    for eg in range(2):
                        op_ps = ps_sc.tile([128, 512], F32, name="op_ps", tag="mix")
                        nc.tensor.matmul(
                            op_ps[:],
                            o_sbs[b][:, tt * 128 : (tt + 1) * 128],
                            wp_sb[:, eg * 512 : (eg + 1) * 512],
                            start=True,
                            stop=True,
                        )
                        if eg == 0:
                            nc.scalar.copy(st[:, 0:512], op_ps[:])
                        else:
                            nc.vector.tensor_copy(st[:, 512:1024], op_ps[:])
                    nc.gpsimd.dma_start(
                        out_p[(b * 8 + tt) * 128 : (b * 8 + tt + 1) * 128, :],
                        st[:],
                    )

            # ---- interleaved schedule ----
            proj(wk_sb, bk_sb, k_sb, [0, 1])  # K for b0
            proj(wq_sb, bq_sb, q_sb, [0])  # Q for b0/ng0
            scores_exp(0, 0)
            proj(wq_sb, bq_sb, q_sb, [1])  # Q for b0/ng1
            scores_exp(0, 1)
            proj(wv_sb, bv_sb, vt_sb, [0, 1])  # V for b0
            v_transpose(range(0, 8))
            attnv_epilogue(0, 0)
            proj(wk_sb, bk_sb, k_sb, [2, 3])  # K for b1
            proj(wq_sb, bq_sb, q_sb, [2, 3])  # Q for b1
            attnv_epilogue(0, 1)
            scores_exp(1, 0)
            proj(wv_sb, bv_sb, vt_sb, [2, 3])  # V for b1
            v_transpose(range(8, 16))
            scores_exp(1, 1)
            attnv_epilogue(1, 0)
            out_proj(0, range(0, 8))
            out_proj(1, range(0, 4))
            attnv_epilogue(1, 1)
            out_proj(1, range(4, 8))

    nc.compile()
    _NC_CACHE["nc"] = nc
    return nc


def prepare_in_maps(x, Wq, bq, Wkv, bkv, Wp, bp):
    bf = ml_dtypes.bfloat16
    x = np.asarray(x, np.float32)
    Wq = np.asarray(Wq, np.float32)
    Wkv = np.asarray(Wkv, np.float32)
    Wp = np.asarray(Wp, np.float32)
    bq = np.asarray(bq, np.float32)
    bkv = np.asarray(bkv, np.float32)

    # xt[p, k, t] = x[t, k*128+p]
    xt = np.ascontiguousarray(x.reshape(T, KT, 128).transpose(2, 1, 0)).astype(bf)

    def wprep(Wslice):  # [128 f, C] -> [p, k, f] with c = k*128+p
        return np.ascontiguousarray(
            Wslice.T.reshape(KT, 128, 128).transpose(1, 0, 2)
        ).astype(bf)

    in_maps = []
    for core in range(CORES):
        f0 = core * 128
        in_maps.append(
            {
                "xt": xt,
                "wq": wprep(Wq[f0 : f0 + 128, :]),
                "wk": wprep(Wkv[f0 : f0 + 128, :]),
                "wv": wprep(Wkv[C + f0 : C + f0 + 128, :]),
                "wp": np.ascontiguousarray(Wp[:, f0 : f0 + 128].T).astype(bf),
                "bq": bq[f0 : f0 + 128].reshape(128, 1).astype(np.float32),
                "bk": bkv[f0 : f0 + 128].reshape(128, 1).astype(np.float32),
                "bv": bkv[C + f0 : C + f0 + 128].reshape(128, 1).astype(np.float32),
            }
        )
    return in_maps


def gather_outputs(results, bp):
    bp = np.asarray(bp, np.float32)
    attn = np.empty((B, N, N, H), np.float32)
    out = np.zeros((T, C), np.float32)
    for core in range(CORES):
        r = results[core]
        a = r["attn_o"]  # [B, 2, m, n] bf16
        for hl in range(2):
            attn[:, :, :, core * 2 + hl] = a[:, hl].astype(np.float32).transpose(
                0, 2, 1
            )
        out += r["out_p"].astype(np.float32)
    out += bp[None, :]
    return out.reshape(B, N, C), attn


def kernel(x, Wq, bq, Wkv, bkv, Wp, bp):
    nc = build_nc()
    in_maps = prepare_in_maps(x, Wq, bq, Wkv, bkv, Wp, bp)
    res = run_bass_kernel_spmd(nc, in_maps, core_ids=list(range(CORES)))
    return gather_outputs(res.results, bp)


# revision 50
# speedup vs baseline: 1.2515x; 1.0269x over previous
"""Multi-head attention Trainium2 kernel, SPMD over 8 NeuronCores.

Problem: x:(2,1024,1024), Wq:(1024,1024), Wkv:(2048,1024), Wp:(1024,1024).
reference returns (out, attn) with attn:(b, n, m, h) softmax probabilities.

Sharding: tensor-parallel over heads — 2 heads (= a 128-wide feature slice)
per core. Each core computes Q^T/K^T/V for its heads over all tokens, the
full attention for its heads (writing its slice of `attn`), and a partial
output projection (row-sharded Wp); partials are summed on the host during
the gather.

All matmuls run in bf16 (fp32 PSUM accumulation). The softmax runs without
max-subtraction (|scores*scale| <~ 6 for this problem family; exp is safe in
fp32): E = exp(S/8) on ScalarE straight out of PSUM. The attn@V matmul uses
a 65-column V_aug = [V_h | ones] so its PSUM accumulator also carries the
softmax row-sums D (row 64). Normalization uses reciprocal_approx_fast on a
partition-broadcast of D and DVE multiplies.

The emission order interleaves projections with attention groups so the
ScalarE exp stream starts ~10us in and every engine stays busy:
K(b0) -> Q(b0) -> scores/exp(b0) -> V -> attn@V(b0) -> K/Q(b1) -> ...
"""

import numpy as np
import ml_dtypes

import concourse.bass as bass
import concourse.tile as tile
import concourse.mybir as mybir
from concourse import bacc
from concourse.bass_utils import run_bass_kernel_spmd
from concourse.masks import make_identity

BF16 = mybir.dt.bfloat16
F32 = mybir.dt.float32
AF = mybir.ActivationFunctionType

B = 2
N = 1024  # tokens per batch
C = 1024  # model dim
H = 16
DH = 64
CORES = 8
T = B * N  # 2048 flattened tokens
KT = C // 128  # 8 contraction tiles
SCALE = DH**-0.5  # 0.125

_NC_CACHE = {}


def build_nc():
    if "nc" in _NC_CACHE:
        return _NC_CACHE["nc"]
    nc = bacc.Bacc("TRN2", target_bir_lowering=False, debug=False, num_devices=CORES)

    # inputs come pre-laid-out from the host so every load is a contiguous
    # [128, X] DMA: xt[p, k, t] = x[t, k*128+p], w*[p, k, f] = W[f, k*128+p]
    xt = nc.declare_dram_parameter("xt", [128, KT, T], BF16, isOutput=False)
    wq = nc.declare_dram_parameter("wq", [128, KT, 128], BF16, isOutput=False)
    wk = nc.declare_dram_parameter("wk", [128, KT, 128], BF16, isOutput=False)
    wv = nc.declare_dram_parameter("wv", [128, KT, 128], BF16, isOutput=False)
    wp = nc.declare_dram_parameter("wp", [128, C], BF16, isOutput=False)
    bq = nc.declare_dram_parameter("bq", [128, 1], F32, isOutput=False)
    bk = nc.declare_dram_parameter("bk", [128, 1], F32, isOutput=False)
    bv = nc.declare_dram_parameter("bv", [128, 1], F32, isOutput=False)
    # attn slice for this core's 2 heads, stored (b, h_local, m, n)
    attn_o = nc.declare_dram_parameter("attn_o", [B, 2, N, N], BF16, isOutput=True)
    # partial output projection (t, e); host sums over cores
    out_p = nc.declare_dram_parameter("out_p", [T, C], BF16, isOutput=True)

    with tile.TileContext(nc) as tc:
        with (
            tc.tile_pool(name="const", bufs=1) as constp,
            tc.tile_pool(name="xw", bufs=1) as xw,
            tc.tile_pool(name="qkv", bufs=1) as qkv,
            tc.tile_pool(name="epool", bufs=4) as epool,
            tc.tile_pool(name="norm", bufs=3) as norm,
            tc.tile_pool(name="obuf", bufs=1) as obuf,
            tc.tile_pool(name="stage", bufs=4) as stage,
            tc.tile_pool(name="ps_sc", bufs=2, space="PSUM") as ps_sc,
            tc.tile_pool(name="ps_acc", bufs=1, space="PSUM") as ps_acc,
        ):
            # ---- input loads (all contiguous [128, X] descriptors) ----
            wk_sb = xw.tile([128, KT, 128], BF16)
            nc.sync.dma_start(wk_sb[:], wk[:])
            wq_sb = xw.tile([128, KT, 128], BF16)
            nc.scalar.dma_start(wq_sb[:], wq[:])
            bq_sb = xw.tile([128, 1], F32)
            nc.scalar.dma_start(bq_sb[:], bq[:])
            bk_sb = xw.tile([128, 1], F32)
            nc.scalar.dma_start(bk_sb[:], bk[:])
            xt_sb = xw.tile([128, KT, T], BF16)
            for k in range(KT):
                eng = (nc.sync, nc.scalar, nc.gpsimd)[k % 3]
                eng.dma_start(xt_sb[:, k, :], xt[:, k, :])
            wv_sb = xw.tile([128, KT, 128], BF16)
            nc.gpsimd.dma_start(wv_sb[:], wv[:])
            bv_sb = xw.tile([128, 1], F32)
            nc.gpsimd.dma_start(bv_sb[:], bv[:])
            wp_sb = xw.tile([128, C], BF16)
            nc.gpsimd.dma_start(wp_sb[:], wp[:])

            # ---- constants (after DMA issues so they don't delay the
            # gpsimd-ring input loads; memsets on the idle DVE) ----
            ident = constp.tile([128, 128], BF16)
            nc.vector.memset(ident[:], 0.0)
            nc.gpsimd.affine_select(
                out=ident[:],
                in_=ident[:],
                compare_op=mybir.AluOpType.not_equal,
                fill=1.0,
                base=0,
                pattern=[[-1, 128]],
                channel_multiplier=1,
            )
            ones_bc = constp.tile([128, 128], BF16)
            nc.vector.memset(ones_bc[:], 1.0)

            q_sb = qkv.tile([128, T], BF16)
            k_sb = qkv.tile([128, T], BF16)
            vt_sb = qkv.tile([128, T], BF16)  # V^T (dv, t), pre-transpose
            # V_aug per m-tile: [V_h0(64) | ones | V_h1(64) | ones] so the
            # attn@V matmul (M=65) also accumulates the softmax row-sums D
            v_sb = qkv.tile([128, 16, 130], BF16)
            nc.vector.memset(v_sb[:], 1.0)
            o_sbs = [obuf.tile([128, N], BF16, name=f"o_sb{b}") for b in range(B)]

            def proj(w_t, b_t, dst, tgs):
                """project tile-groups tgs of tokens into dst (feat x T).

                k is the outer loop so each xt chunk is consumed as soon as
                its DMA lands (the QKV phase streams against the x load).
                """
                psqs = [
                    ps_sc.tile([128, 512], F32, name=f"qkv_ps{i}", tag="mix")
                    for i in range(len(tgs))
                ]
                for k in range(KT):
                    for i, tg in enumerate(tgs):
                        nc.tensor.matmul(
                            psqs[i][:],
                            w_t[:, k, :],
                            xt_sb[:, k, tg * 512 : (tg + 1) * 512],
                            start=(k == 0),
                            stop=(k == KT - 1),
                        )
                for i, tg in enumerate(tgs):
                    nc.vector.tensor_scalar_add(
                        dst[:, tg * 512 : (tg + 1) * 512], psqs[i][:], b_t[:]
                    )

            def v_transpose(tiles):
                for i in tiles:
                    vt_ps = ps_sc.tile([128, 128], BF16, name="vt_ps", tag="mix")
                    nc.tensor.transpose(
                        vt_ps[:], vt_sb[:, i * 128 : (i + 1) * 128], ident[:]
                    )
                    nc.vector.tensor_copy(v_sb[:, i, 0:64], vt_ps[:, 0:64])
                    nc.vector.tensor_copy(v_sb[:, i, 65:129], vt_ps[:, 64:128])

            e_ts = {}

            def scores_exp(b, ng):
                """scores + exp for all 8 m-tiles of group (b, ng)."""
                n0 = b * N + ng * 512
                e_t = epool.tile([128, 8, 1024], BF16, name="e_sb")
                e_ts[(b, ng)] = e_t
                for mt in range(8):
                    m0 = b * 8 + mt
                    scp = ps_sc.tile([128, 1024], F32, name="sc_ps")
                    for hl in range(2):
                        nc.tensor.matmul(
                            scp[:, hl * 512 : (hl + 1) * 512],
                            k_sb[hl * 64 : (hl + 1) * 64, m0 * 128 : (m0 + 1) * 128],
                            q_sb[hl * 64 : (hl + 1) * 64, n0 : n0 + 512],
                            start=True,
                            stop=True,
                        )
                    nc.scalar.activation(e_t[:, mt, :], scp[:], AF.Exp, scale=SCALE)

            def attnv_epilogue(b, ng):
                """attn@V + row-sum extraction + normalize + writes."""
                e_t = e_ts[(b, ng)]
                o0 = ps_acc.tile([65, 512], F32, name="o0")
                o1 = ps_acc.tile([65, 512], F32, name="o1")
                for mt in range(8):
                    m0 = b * 8 + mt
                    first, last = mt == 0, mt == 7
                    nc.tensor.matmul(
                        o0[:],
                        v_sb[:, m0, 0:65],
                        e_t[:, mt, 0:512],
                        start=first,
                        stop=last,
                    )
                    nc.tensor.matmul(
                        o1[:],
                        v_sb[:, m0, 65:130],
                        e_t[:, mt, 512:1024],
                        start=first,
                        stop=last,
                    )
                # extract D rows (psum part 64) to a base-0 partition,
                # broadcast to all 128, reciprocal
                dsb = norm.tile([128, 1024], BF16, name="dsb")
                nc.vector.tensor_copy(dsb[64:65, 0:512], o0[64:65, :])
                nc.vector.tensor_copy(dsb[64:65, 512:1024], o1[64:65, :])
                # broadcast D to all 128 partitions with a K=1 ones matmul
                # (lhsT sliced at base 64 to lane-align with the D rows)
                db_ps0 = ps_sc.tile([128, 512], F32, name="db_ps0", tag="mix")
                nc.tensor.matmul(
                    db_ps0[:], ones_bc[64:65, :], dsb[64:65, 0:512],
                    start=True, stop=True,
                )
                db_ps1 = ps_sc.tile([128, 512], F32, name="db_ps1", tag="mix")
                nc.tensor.matmul(
                    db_ps1[:], ones_bc[64:65, :], dsb[64:65, 512:1024],
                    start=True, stop=True,
                )
                rb = norm.tile([128, 1024], F32, name="rb")
                nc.vector.reciprocal_approx_fast(out=rb[:, 0:512], in_=db_ps0[:])
                nc.vector.reciprocal_approx_fast(out=rb[:, 512:1024], in_=db_ps1[:])
                rbb = norm.tile([128, 1024], BF16, name="rbb")
                nc.vector.tensor_copy(rbb[:], rb[:])
                # O^T scaled: o_sb rows 0:64 <- h0; h1 lands at psum parts
                # 0:64, scaled into a staging tile then DMA-shifted to rows
                # 64:128
                nc.vector.tensor_mul(
                    o_sbs[b][0:64, ng * 512 : (ng + 1) * 512],
                    o0[0:64, :],
                    rb[0:64, 0:512],
                )
                otmp = norm.tile([64, 512], BF16, name="otmp")
                nc.vector.tensor_mul(otmp[:], o1[0:64, :], rb[0:64, 512:1024])
                nc.scalar.dma_start(
                    o_sbs[b][64:128, ng * 512 : (ng + 1) * 512], otmp[:]
                )
                # normalize E in place (both heads in one op per mt) and
                # write the attn slice (both heads in one DMA per mt)
                for mt in range(8):
                    nc.vector.tensor_mul(e_t[:, mt, :], e_t[:, mt, :], rbb[:])
                    dst = attn_o[
                        b, :, mt * 128 : (mt + 1) * 128, ng * 512 : (ng + 1) * 512
                    ].rearrange("h m n -> m h n")
                    src = e_t[:, mt, :].rearrange("m (h n) -> m h n", h=2)
                    nc.sync.dma_start(dst, src)

            def out_proj(b, tts):
                """partial output projection for token tiles tts of batch b.
                Tile tt only needs o_sb columns tt*128:(tt+1)*128, so the
                first half can run right after the (b, ng=0) epilogue."""
                for tt in tts:
                    st = stage.tile([128, 1024], BF16, name="op_st")
                    for eg in range(2):
                        op_ps = ps_sc.tile([128, 512], F32, name="op_ps", tag="mix")
                        nc.tensor.matmul(
                            op_ps[:],
                            o_sbs[b][:, tt * 128 : (tt + 1) * 128],
                            wp_sb[:, eg * 512 : (eg + 1) * 512],
                            start=True,
                            stop=True,
                        )
                        if eg == 0:
                            nc.scalar.copy(st[:, 0:512], op_ps[:])
                        else:
                            nc.vector.tensor_copy(st[:, 512:1024], op_ps[:])
                    nc.gpsimd.dma_start(
                        out_p[(b * 8 + tt) * 128 : (b * 8 + tt + 1) * 128, :],
                        st[:],
                    )

            # ---- interleaved schedule ----
            proj(wk_sb, bk_sb, k_sb, [0, 1])  # K for b0
            proj(wq_sb, bq_sb, q_sb, [0])  # Q for b0/ng0
            scores_exp(0, 0)
            proj(wq_sb, bq_sb, q_sb, [1])  # Q for b0/ng1
            scores_exp(0, 1)
            proj(wv_sb, bv_sb, vt_sb, [0, 1])  # V for b0
            v_transpose(range(0, 8))
            attnv_epilogue(0, 0)
            proj(wk_sb, bk_sb, k_sb, [2, 3])  # K for b1
            proj(wq_sb, bq_sb, q_sb, [2, 3])  # Q for b1
            attnv_epilogue(0, 1)
            scores_exp(1, 0)
            proj(wv_sb, bv_sb, vt_sb, [2, 3])  # V for b1
            v_transpose(range(8, 16))
            scores_exp(1, 1)
            attnv_epilogue(1, 0)
            out_proj(0, range(0, 8))
            out_proj(1, range(0, 4))
            attnv_epilogue(1, 1)
            out_proj(1, range(4, 8))

    nc.compile()
    _NC_CACHE["nc"] = nc
    return nc


def prepare_in_maps(x, Wq, bq, Wkv, bkv, Wp, bp):
    bf = ml_dtypes.bfloat16
    x = np.asarray(x, np.float32)
    Wq = np.asarray(Wq, np.float32)
    Wkv = np.asarray(Wkv, np.float32)
    Wp = np.asarray(Wp, np.float32)
    bq = np.asarray(bq, np.float32)
    bkv = np.asarray(bkv, np.float32)

    # xt[p, k, t] = x[t, k*128+p]
    xt = np.ascontiguousarray(x.reshape(T, KT, 128).transpose(2, 1, 0)).astype(bf)

    def wprep(Wslice):  # [128 f, C] -> [p, k, f] with c = k*128+p
        return np.ascontiguousarray(
            Wslice.T.reshape(KT, 128, 128).transpose(1, 0, 2)
        ).astype(bf)

    in_maps = []
    for core in range(CORES):
        f0 = core * 128
        in_maps.append(
            {
                "xt": xt,
                "wq": wprep(Wq[f0 : f0 + 128, :]),
                "wk": wprep(Wkv[f0 : f0 + 128, :]),
                "wv": wprep(Wkv[C + f0 : C + f0 + 128, :]),
                "wp": np.ascontiguousarray(Wp[:, f0 : f0 + 128].T).astype(bf),
                "bq": bq[f0 : f0 + 128].reshape(128, 1).astype(np.float32),
                "bk": bkv[f0 : f0 + 128].reshape(128, 1).astype(np.float32),
                "bv": bkv[C + f0 : C + f0 + 128].reshape(128, 1).astype(np.float32),
            }
        )
    return in_maps


def gather_outputs(results, bp):
    bp = np.asarray(bp, np.float32)
    attn = np.empty((B, N, N, H), np.float32)
    out = np.zeros((T, C), np.float32)
    for core in range(CORES):
        r = results[core]
        a = r["attn_o"]  # [B, 2, m, n] bf16
        for hl in range(2):
            attn[:, :, :, core * 2 + hl] = a[:, hl].astype(np.float32).transpose(
                0, 2, 1
            )
        out += r["out_p"].astype(np.float32)
    out += bp[None, :]
    return out.reshape(B, N, C), attn


def kernel(x, Wq, bq, Wkv, bkv, Wp, bp):
    nc = build_nc()
    in_maps = prepare_in_maps(x, Wq, bq, Wkv, bkv, Wp, bp)
    res = run_bass_kernel_spmd(nc, in_maps, core_ids=list(range(CORES)))
    return gather_outputs(res.results, bp)
